# revision 10
# baseline (speedup 1.0000x reference)
"""GCN block (DGL GraphConv norm='both' + ReLU) on 8 TRN2 NeuronCores.

Strategy (SPMD, one program for all cores; per-core data via inputs):
  - Nodes/edges sharded by destination: core c owns dst rows [c*6250, (c+1)*6250).
  - The gather table is the raw bf16 x (no normalized-table build pass):
    the src normalization rsqrt(deg_out) is folded into the one-hot rhs of
    the segment-sum matmul. The one-hot is built by DVE (is_equal against a
    128-iota) and scaled per-tile by the Scalar engine (per-partition scale
    AP), keeping both off the critical path.
  - Edges sorted by (src half, dst window of 128). Per (window, half) group
    the tile count is the max over the 8 cores (SPMD uniform schedule);
    within a group each 128-edge tile does one matmul
    psum[128f, 128d] += g[128e, 128f]^T @ oh[128e, 128d].
  - After each group, psum is copied/added into the per-window aggW tile.
    The per-dst-chunk output (aggW^T W, * rsqrt(deg_in), ReLU) is emitted
    inline as soon as a window's last group completes, so the output stage
    overlaps the gather stream (bias is fused away when b == 0, which is
    checked on the host).

dma_gather indices are int16, so the table is split in two halves at row
32768; edges are grouped into two passes by source half. GpSimd descriptor
generation (~3.1 ns/idx, serial on the Pool engine) is the bottleneck; the
schedule exists to keep every other engine hidden under it.
"""

import sys

if "/opt/trn_rl_repo" not in sys.path:
    sys.path.insert(0, "/opt/trn_rl_repo")

import numpy as np
import ml_dtypes

import concourse.bacc as bacc
import concourse.mybir as mybir
from concourse.bass import AP
from concourse.bass_utils import run_bass_kernel_spmd
from concourse.tile import TileContext

N = 50000          # nodes
D = 128            # feature dim
NCORES = 8
NPC = N // NCORES  # 6250 dst nodes per core

RN = 50048         # padded node count (multiple of 128)
HALF = 32768       # int16 index limit; table split [0, HALF) / [HALF, RN)

WND = 128                         # dst window width (= psum cols per group)
NW = (NPC + WND - 1) // WND       # 49 windows per core
OCH = NW                          # output chunks of 128 dst rows

GCH = 32                          # gather chunk: tiles per dma_gather call
NQ = 4                            # SWDGE queues used round-robin

F32 = mybir.dt.float32
BF16 = mybir.dt.bfloat16
I16 = mybir.dt.int16

TRACE = False            # set by test harness for profiling
LAST_RESULTS = None      # BassKernelResults of the last run


def _gather_idx_layout(vals):
    """[E] int16 -> [128, E//16] in dma_gather layout (16-wrap, 8x replicated)."""
    base = vals.reshape(-1, 16).T          # [16, E/16]
    return np.ascontiguousarray(np.tile(base, (8, 1)))


def _prep_inputs(x, edge_index, W, b):
    src = np.asarray(edge_index[0], dtype=np.int64)
    dst = np.asarray(edge_index[1], dtype=np.int64)
    E = src.shape[0]

    deg_out = np.bincount(src, minlength=N).astype(np.float64)
    deg_in = np.bincount(dst, minlength=N).astype(np.float64)
    ns = (1.0 / np.sqrt(np.maximum(deg_out, 1.0))).astype(np.float32)  # [N]
    nd = (1.0 / np.sqrt(np.maximum(deg_in, 1.0))).astype(np.float32)   # [N]

    core = dst // NPC
    dstl = dst - core * NPC
    half = (src >= HALF).astype(np.int64)
    w = dstl // WND

    # group id per (core, half, window); emit order is half-major
    gid = (core * 2 + half) * NW + w
    counts = np.bincount(gid, minlength=NCORES * 2 * NW).reshape(NCORES, 2 * NW)
    # uniform tiles per (half, window) group across cores
    T = np.maximum(0, -(-counts.max(axis=0) // 128)).astype(np.int64)  # [2*NW]
    tile_base = np.zeros(2 * NW + 1, dtype=np.int64)
    np.cumsum(T, out=tile_base[1:])
    TT = int(tile_base[-1])          # total tiles per core
    ThA = int(T[:NW].sum())          # tiles in half-0 pass
    ThB = TT - ThA

    # slot assignment: per core, edges ranked within their group
    order = np.argsort(gid, kind="stable")
    gid_s = gid[order]
    gstart = np.zeros(NCORES * 2 * NW + 1, dtype=np.int64)
    np.cumsum(counts.reshape(-1), out=gstart[1:])
    rank = np.arange(E, dtype=np.int64) - gstart[gid_s]

    core_s = core[order]
    slot = tile_base[gid_s - core_s * 2 * NW] * 128 + rank  # slot in core's schedule
    src_s = src[order]
    half_s = half[order]
    dl_s = (dstl - w * WND)[order]
    ns_s = ns[src_s]

    NSLOT = TT * 128
    idx_all = np.zeros((NCORES, NSLOT), dtype=np.int16)
    dl_all = np.full((NCORES, NSLOT), -1.0, dtype=np.float32)
    ns_all = np.zeros((NCORES, NSLOT), dtype=np.float32)
    idx_all[core_s, slot] = np.where(half_s == 0, src_s, src_s - HALF).astype(np.int16)
    dl_all[core_s, slot] = dl_s
    ns_all[core_s, slot] = ns_s
    dl_all = dl_all.astype(ml_dtypes.bfloat16)

    # tile meta shared by all cores: (window, k within group, group size)
    tile_meta = []
    for g in range(2 * NW):
        for k in range(int(T[g])):
            tile_meta.append((g % NW, k, int(T[g])))

    bias_zero = bool(np.all(np.asarray(b) == 0.0))

    # replicated tensors
    xp = np.zeros((RN, D), dtype=ml_dtypes.bfloat16)
    xp[:N] = np.asarray(x, dtype=np.float32).astype(ml_dtypes.bfloat16)
    x_dev = np.ascontiguousarray(xp)

    W_dev = np.ascontiguousarray(np.asarray(W, dtype=np.float32))
    brep = np.ascontiguousarray(
        np.tile(np.asarray(b, dtype=np.float32)[None, :], (128, 1)))
    iota = np.ascontiguousarray(
        np.tile(np.arange(WND, dtype=np.float32)[None, :], (128, 1))
    ).astype(ml_dtypes.bfloat16)

    in_maps = []
    for c in range(NCORES):
        ndp = np.zeros(OCH * 128, dtype=np.float32)
        ndp[:NPC] = nd[c * NPC:(c + 1) * NPC]
        nd_dev = np.ascontiguousarray(ndp.reshape(OCH, 128).T)  # [128, OCH]
        in_maps.append({
            "x_dev": x_dev,
            "ndr": nd_dev,
            "w": W_dev,
            "brep": brep,
            "iota": iota,
            "idx_a": _gather_idx_layout(idx_all[c, :ThA * 128]),
            "idx_b": _gather_idx_layout(idx_all[c, ThA * 128:]),
            # per tile: [128, TT], tile t partition p = edge slot t*128+p
            "dl": np.ascontiguousarray(dl_all[c].reshape(TT, 128).T),
            "nse": np.ascontiguousarray(ns_all[c].reshape(TT, 128).T),
        })
    return in_maps, tile_meta, ThA, ThB, bias_zero


def _build_program(tile_meta, ThA, ThB, bias_zero):
    TT = ThA + ThB
    # tiles per (half, window) group, recovered from tile_meta
    TB = [0] * NW
    for t in range(ThA, TT):
        TB[tile_meta[t][0]] += 1

    nc = bacc.Bacc("TRN2", target_bir_lowering=False, debug=False,
                   num_devices=NCORES, num_swdge_queues=NQ)

    x_d = nc.dram_tensor("x_dev", [RN, D], BF16, kind="ExternalInput")
    ndr_d = nc.dram_tensor("ndr", [128, OCH], F32, kind="ExternalInput")
    w_d = nc.dram_tensor("w", [D, D], F32, kind="ExternalInput")
    brep_d = nc.dram_tensor("brep", [128, D], F32, kind="ExternalInput")
    iota_d = nc.dram_tensor("iota", [128, WND], BF16, kind="ExternalInput")
    idx_a = nc.dram_tensor("idx_a", [128, ThA * 8], I16, kind="ExternalInput")
    idx_b = nc.dram_tensor("idx_b", [128, ThB * 8], I16, kind="ExternalInput")
    dl_d = nc.dram_tensor("dl", [128, TT], BF16, kind="ExternalInput")
    nse_d = nc.dram_tensor("nse", [128, TT], F32, kind="ExternalInput")
    y_d = nc.dram_tensor("y", [128, OCH, D], F32, kind="ExternalOutput")

    with TileContext(nc) as tc:
        with (
            tc.tile_pool(name="const", bufs=1) as cpool,
            tc.tile_pool(name="gbuf", bufs=4) as gpool,
            tc.tile_pool(name="ohbuf", bufs=4) as opool,
            tc.tile_pool(name="agg", bufs=1) as apool,
            tc.tile_pool(name="psum", bufs=6, space="PSUM") as ppool,
            tc.tile_pool(name="psum2", bufs=2, space="PSUM") as ppool2,
        ):
            # ---- constants / small loads ----
            idx_a_sb = cpool.tile([128, ThA * 8], I16, tag="idxa")
            nc.sync.dma_start(out=idx_a_sb[:], in_=idx_a[:, :])
            idx_b_sb = cpool.tile([128, ThB * 8], I16, tag="idxb")
            nc.sync.dma_start(out=idx_b_sb[:], in_=idx_b[:, :])
            dl_sb = cpool.tile([128, TT], BF16, tag="dl")
            nc.sync.dma_start(out=dl_sb[:], in_=dl_d[:, :])
            nse_sb = cpool.tile([128, TT], F32, tag="nse")
            nc.sync.dma_start(out=nse_sb[:], in_=nse_d[:, :])
            iota_sb = cpool.tile([128, WND], BF16, tag="iota")
            nc.sync.dma_start(out=iota_sb[:], in_=iota_d[:, :])
            w_sb = cpool.tile([D, D], F32, tag="w")
            nc.sync.dma_start(out=w_sb[:], in_=w_d[:, :])
            brep_sb = cpool.tile([128, D], F32, tag="brep")
            nc.sync.dma_start(out=brep_sb[:], in_=brep_d[:, :])
            ndr_sb = cpool.tile([128, OCH], F32, tag="ndr")
            nc.sync.dma_start(out=ndr_sb[:], in_=ndr_d[:, :])

            aggW = [apool.tile([128, WND], F32, tag=f"agg{w}", name=f"aggW{w}")
                    for w in range(NW)]
            outall = apool.tile([128, OCH, D], F32, tag="outall")
            touched = [False] * NW
            out_done = [False] * NW

            def emit_output(wdw):
                ps2 = ppool2.tile([128, D], F32, tag="ps2")
                nc.tensor.matmul(
                    ps2[:],
                    lhsT=aggW[wdw][:],
                    rhs=w_sb[:],
                    start=True,
                    stop=True,
                )
                if bias_zero:
                    # out = relu(ps2 * nd)
                    nc.vector.tensor_scalar(
                        outall[:, wdw, :], ps2[:], ndr_sb[:, wdw:wdw + 1], 0.0,
                        mybir.AluOpType.mult, mybir.AluOpType.max,
                    )
                else:
                    nc.vector.tensor_scalar(
                        outall[:, wdw, :], ps2[:], ndr_sb[:, wdw:wdw + 1], None,
                        mybir.AluOpType.mult,
                    )
                    nc.vector.tensor_tensor(
                        outall[:, wdw, :], outall[:, wdw, :], brep_sb[:],
                        mybir.AluOpType.add,
                    )
                    nc.vector.tensor_scalar_max(
                        outall[:, wdw, :], outall[:, wdw, :], 0.0)
                out_done[wdw] = True

            qn = 0
            psum = None
            for is_b, idx_sb, Th, base_t, h_ap in (
                (False, idx_a_sb, ThA, 0, x_d[0:HALF, :]),
                (True, idx_b_sb, ThB, ThA, x_d[HALF:RN, :]),
            ):
                for t0 in range(0, Th, GCH):
                    nt = min(GCH, Th - t0)
                    nidx = nt * 128
                    g = gpool.tile([128, GCH, D], BF16, tag="g")
                    nc.gpsimd.dma_gather(
                        g[:, :nt, :],
                        h_ap,
                        idx_sb[:, t0 * 8:t0 * 8 + nidx // 16],
                        num_idxs=nidx,
                        num_idxs_reg=nidx,
                        elem_size=D,
                        single_packet=False,
                        queue_num=qn % NQ,
                    )
                    qn += 1
                    gt0 = base_t + t0
                    oh = opool.tile([128, GCH, WND], BF16, tag="oh")
                    nc.vector.tensor_tensor(
                        oh[:, :nt, :],
                        dl_sb[:, gt0:gt0 + nt, None].to_broadcast([128, nt, WND]),
                        iota_sb[:, None, :].to_broadcast([128, nt, WND]),
                        mybir.AluOpType.is_equal,
                    )
                    for tl in range(nt):
                        t = gt0 + tl
                        wdw, k, Twh = tile_meta[t]
                        # fold ns[src] into the one-hot on the Scalar engine
                        nc.scalar.mul(
                            oh[:, tl, :], oh[:, tl, :], nse_sb[:, t:t + 1])
                        if k == 0:
                            psum = ppool.tile([128, WND], F32, tag="ps")
                        nc.tensor.matmul(
                            psum[:],
                            lhsT=g[:, tl, :],
                            rhs=oh[:, tl, :],
                            start=(k == 0),
                            stop=(k == Twh - 1),
                        )
                        if k == Twh - 1:
                            if not touched[wdw]:
                                nc.vector.tensor_copy(aggW[wdw][:], psum[:])
                                touched[wdw] = True
                            else:
                                nc.vector.tensor_add(
                                    aggW[wdw][:], aggW[wdw][:], psum[:])
                            if is_b:
                                emit_output(wdw)

            for wdw in range(NW):
                if not touched[wdw]:
                    nc.vector.memset(aggW[wdw][:], 0.0)
                if not out_done[wdw]:
                    emit_output(wdw)

            nc.sync.dma_start(out=y_d[:, :, :], in_=outall[:])

    nc.compile()
    return nc


def kernel(x, edge_index, W, b):
    global LAST_RESULTS
    x = np.asarray(x, dtype=np.float32)
    W = np.asarray(W, dtype=np.float32)
    b = np.asarray(b, dtype=np.float32)

    in_maps, tile_meta, ThA, ThB, bias_zero = _prep_inputs(x, edge_index, W, b)
    nc = _build_program(tile_meta, ThA, ThB, bias_zero)

    kwargs = {}
    if TRACE:
        kwargs["trace"] = True
    res = run_bass_kernel_spmd(nc, in_maps, list(range(NCORES)), **kwargs)
    LAST_RESULTS = res

    out = np.empty((N, D), dtype=np.float32)
    for c in range(NCORES):
        yc = np.asarray(res.results[c]["y"])          # [128, OCH, 128]
        rows = yc.transpose(1, 0, 2).reshape(OCH * 128, D)
        out[c * NPC:(c + 1) * NPC] = rows[:NPC]
    return out


# revision 11
# speedup vs baseline: 1.1163x; 1.1163x over previous
"""GCN block (DGL GraphConv norm='both' + ReLU) on 8 TRN2 NeuronCores.

Strategy (SPMD, one program for all cores; per-core data via inputs):
  - Nodes/edges sharded by destination: core c owns dst rows [c*6250, (c+1)*6250).
  - The gather table is the raw bf16 x (no normalized-table build pass):
    the src normalization rsqrt(deg_out) is folded into the one-hot rhs of
    the segment-sum matmul (DVE is_equal against an iota, then a batched
    DVE multiply by ns[src] per edge slot).
  - Edges sorted by (src half, dst window of 128). Per (window, half) group
    the tile count is the max over the 8 cores (SPMD uniform schedule);
    within a group each 128-edge tile does one matmul
    psum[128f, 128d] += g[128e, 128f]^T @ oh[128e, 128d].
  - After each group, psum is copied (Scalar engine) / added (DVE) into the
    per-window aggW tile. As soon as a window's last group completes, the
    output chunk runs inline: PE matmul aggW^T @ W, Scalar engine
    Relu(psum * rsqrt(deg_in)) (bias fused away when b == 0), and a
    per-window DMA of the 128 output rows, so the output stage fully
    overlaps the gather stream.

dma_gather indices are int16, so the table is split in two halves at row
32768; edges are grouped into two passes by source half. GpSimd descriptor
generation (~3 ns/idx, strictly serial on the Pool engine) is the
bottleneck; the schedule exists to keep every other engine hidden under it.
"""

import sys

if "/opt/trn_rl_repo" not in sys.path:
    sys.path.insert(0, "/opt/trn_rl_repo")

import numpy as np
import ml_dtypes

import concourse.bacc as bacc
import concourse.mybir as mybir
from concourse.bass import AP
from concourse.bass_utils import run_bass_kernel_spmd
from concourse.tile import TileContext

N = 50000          # nodes
D = 128            # feature dim
NCORES = 8
NPC = N // NCORES  # 6250 dst nodes per core

RN = 50048         # padded node count (multiple of 128)
HALF = 32768       # int16 index limit; table split [0, HALF) / [HALF, RN)

WND = 128                         # dst window width (= psum cols per group)
NW = (NPC + WND - 1) // WND       # 49 windows per core
OCH = NW                          # output chunks of 128 dst rows

GCH = 16                          # gather chunk: tiles per dma_gather call
NQ = 4                            # SWDGE queues used round-robin

F32 = mybir.dt.float32
BF16 = mybir.dt.bfloat16
I16 = mybir.dt.int16

TRACE = False            # set by test harness for profiling
LAST_RESULTS = None      # BassKernelResults of the last run


def _gather_idx_layout(vals):
    """[E] int16 -> [128, E//16] in dma_gather layout (16-wrap, 8x replicated)."""
    base = vals.reshape(-1, 16).T          # [16, E/16]
    return np.ascontiguousarray(np.tile(base, (8, 1)))


def _prep_inputs(x, edge_index, W, b):
    src = np.asarray(edge_index[0], dtype=np.int64)
    dst = np.asarray(edge_index[1], dtype=np.int64)
    E = src.shape[0]

    deg_out = np.bincount(src, minlength=N).astype(np.float64)
    deg_in = np.bincount(dst, minlength=N).astype(np.float64)
    ns = (1.0 / np.sqrt(np.maximum(deg_out, 1.0))).astype(np.float32)  # [N]
    nd = (1.0 / np.sqrt(np.maximum(deg_in, 1.0))).astype(np.float32)   # [N]

    core = dst // NPC
    dstl = dst - core * NPC
    half = (src >= HALF).astype(np.int64)
    w = dstl // WND

    # group id per (core, half, window); emit order is half-major
    gid = (core * 2 + half) * NW + w
    counts = np.bincount(gid, minlength=NCORES * 2 * NW).reshape(NCORES, 2 * NW)
    # uniform tiles per (half, window) group across cores
    T = np.maximum(0, -(-counts.max(axis=0) // 128)).astype(np.int64)  # [2*NW]
    tile_base = np.zeros(2 * NW + 1, dtype=np.int64)
    np.cumsum(T, out=tile_base[1:])
    TT = int(tile_base[-1])          # total tiles per core
    ThA = int(T[:NW].sum())          # tiles in half-0 pass
    ThB = TT - ThA

    # slot assignment: per core, edges ranked within their group
    order = np.argsort(gid, kind="stable")
    gid_s = gid[order]
    gstart = np.zeros(NCORES * 2 * NW + 1, dtype=np.int64)
    np.cumsum(counts.reshape(-1), out=gstart[1:])
    rank = np.arange(E, dtype=np.int64) - gstart[gid_s]

    core_s = core[order]
    slot = tile_base[gid_s - core_s * 2 * NW] * 128 + rank  # slot in core's schedule
    src_s = src[order]
    half_s = half[order]
    dl_s = (dstl - w * WND)[order]
    ns_s = ns[src_s]

    NSLOT = TT * 128
    idx_all = np.zeros((NCORES, NSLOT), dtype=np.int16)
    dl_all = np.full((NCORES, NSLOT), -1.0, dtype=np.float32)
    ns_all = np.zeros((NCORES, NSLOT), dtype=np.float32)
    idx_all[core_s, slot] = np.where(half_s == 0, src_s, src_s - HALF).astype(np.int16)
    dl_all[core_s, slot] = dl_s
    ns_all[core_s, slot] = ns_s
    dl_all = dl_all.astype(ml_dtypes.bfloat16)
    ns_all = ns_all.astype(ml_dtypes.bfloat16)

    # tile meta shared by all cores: (window, k within group, group size)
    tile_meta = []
    for g in range(2 * NW):
        for k in range(int(T[g])):
            tile_meta.append((g % NW, k, int(T[g])))

    bias_zero = bool(np.all(np.asarray(b) == 0.0))

    # replicated tensors
    xp = np.zeros((RN, D), dtype=ml_dtypes.bfloat16)
    xp[:N] = np.asarray(x, dtype=np.float32).astype(ml_dtypes.bfloat16)
    x_dev = np.ascontiguousarray(xp)

    W_dev = np.ascontiguousarray(np.asarray(W, dtype=np.float32))
    brep = np.ascontiguousarray(
        np.tile(np.asarray(b, dtype=np.float32)[None, :], (128, 1)))
    iota_rep = np.ascontiguousarray(np.broadcast_to(
        np.arange(WND, dtype=np.float32)[None, None, :], (128, GCH, WND)
    )).astype(ml_dtypes.bfloat16)

    in_maps = []
    for c in range(NCORES):
        ndp = np.zeros(OCH * 128, dtype=np.float32)
        ndp[:NPC] = nd[c * NPC:(c + 1) * NPC]
        nd_dev = np.ascontiguousarray(ndp.reshape(OCH, 128).T)  # [128, OCH]
        in_maps.append({
            "x_dev": x_dev,
            "ndr": nd_dev,
            "w": W_dev,
            "brep": brep,
            "iota_rep": iota_rep,
            "idx_a": _gather_idx_layout(idx_all[c, :ThA * 128]),
            "idx_b": _gather_idx_layout(idx_all[c, ThA * 128:]),
            # per tile: [128, TT], tile t partition p = edge slot t*128+p
            "dl": np.ascontiguousarray(dl_all[c].reshape(TT, 128).T),
            "nse": np.ascontiguousarray(ns_all[c].reshape(TT, 128).T),
        })
    return in_maps, tile_meta, ThA, ThB, bias_zero


def _build_program(tile_meta, ThA, ThB, bias_zero):
    TT = ThA + ThB

    nc = bacc.Bacc("TRN2", target_bir_lowering=False, debug=False,
                   num_devices=NCORES, num_swdge_queues=NQ)

    x_d = nc.dram_tensor("x_dev", [RN, D], BF16, kind="ExternalInput")
    ndr_d = nc.dram_tensor("ndr", [128, OCH], F32, kind="ExternalInput")
    w_d = nc.dram_tensor("w", [D, D], F32, kind="ExternalInput")
    brep_d = nc.dram_tensor("brep", [128, D], F32, kind="ExternalInput")
    iota_d = nc.dram_tensor("iota_rep", [128, GCH, WND], BF16, kind="ExternalInput")
    idx_a = nc.dram_tensor("idx_a", [128, ThA * 8], I16, kind="ExternalInput")
    idx_b = nc.dram_tensor("idx_b", [128, ThB * 8], I16, kind="ExternalInput")
    dl_d = nc.dram_tensor("dl", [128, TT], BF16, kind="ExternalInput")
    nse_d = nc.dram_tensor("nse", [128, TT], BF16, kind="ExternalInput")
    y_d = nc.dram_tensor("y", [128, OCH, D], F32, kind="ExternalOutput")

    with TileContext(nc) as tc:
        with (
            tc.tile_pool(name="const", bufs=1) as cpool,
            tc.tile_pool(name="gbuf", bufs=4) as gpool,
            tc.tile_pool(name="ohbuf", bufs=4) as opool,
            tc.tile_pool(name="agg", bufs=1) as apool,
            tc.tile_pool(name="psum", bufs=6, space="PSUM") as ppool,
            tc.tile_pool(name="psum2", bufs=2, space="PSUM") as ppool2,
        ):
            # ---- constants / small loads ----
            idx_a_sb = cpool.tile([128, ThA * 8], I16, tag="idxa")
            nc.sync.dma_start(out=idx_a_sb[:], in_=idx_a[:, :])
            idx_b_sb = cpool.tile([128, ThB * 8], I16, tag="idxb")
            nc.sync.dma_start(out=idx_b_sb[:], in_=idx_b[:, :])
            dl_sb = cpool.tile([128, TT], BF16, tag="dl")
            nc.sync.dma_start(out=dl_sb[:], in_=dl_d[:, :])
            nse_sb = cpool.tile([128, TT], BF16, tag="nse")
            nc.sync.dma_start(out=nse_sb[:], in_=nse_d[:, :])
            iota_sb = cpool.tile([128, GCH, WND], BF16, tag="iota")
            nc.sync.dma_start(out=iota_sb[:], in_=iota_d[:, :, :])
            w_sb = cpool.tile([D, D], F32, tag="w")
            nc.sync.dma_start(out=w_sb[:], in_=w_d[:, :])
            ndr_sb = cpool.tile([128, OCH], F32, tag="ndr")
            nc.sync.dma_start(out=ndr_sb[:], in_=ndr_d[:, :])
            if not bias_zero:
                brep_sb = cpool.tile([128, D], F32, tag="brep")
                nc.sync.dma_start(out=brep_sb[:], in_=brep_d[:, :])

            aggW = [apool.tile([128, WND], F32, tag=f"agg{w}", name=f"aggW{w}")
                    for w in range(NW)]
            outW = [apool.tile([128, D], F32, tag=f"out{w}", name=f"outW{w}")
                    for w in range(NW)]
            touched = [False] * NW
            out_done = [False] * NW

            def emit_output(wdw):
                ps2 = ppool2.tile([128, D], F32, tag="ps2")
                nc.tensor.matmul(
                    ps2[:],
                    lhsT=aggW[wdw][:],
                    rhs=w_sb[:],
                    start=True,
                    stop=True,
                )
                if bias_zero:
                    # out = relu(ps2 * nd), on the Scalar engine
                    nc.scalar.activation(
                        outW[wdw][:], ps2[:],
                        mybir.ActivationFunctionType.Relu,
                        bias=0.0, scale=ndr_sb[:, wdw:wdw + 1],
                    )
                else:
                    nc.vector.tensor_scalar(
                        outW[wdw][:], ps2[:], ndr_sb[:, wdw:wdw + 1], None,
                        mybir.AluOpType.mult,
                    )
                    nc.vector.tensor_tensor(
                        outW[wdw][:], outW[wdw][:], brep_sb[:],
                        mybir.AluOpType.add,
                    )
                    nc.vector.tensor_scalar_max(outW[wdw][:], outW[wdw][:], 0.0)
                nc.sync.dma_start(out=y_d[:, wdw, :], in_=outW[wdw][:])
                out_done[wdw] = True

            qn = 0
            psum = None
            for is_b, idx_sb, Th, base_t, h_ap in (
                (False, idx_a_sb, ThA, 0, x_d[0:HALF, :]),
                (True, idx_b_sb, ThB, ThA, x_d[HALF:RN, :]),
            ):
                for t0 in range(0, Th, GCH):
                    nt = min(GCH, Th - t0)
                    nidx = nt * 128
                    g = gpool.tile([128, GCH, D], BF16, tag="g")
                    nc.gpsimd.dma_gather(
                        g[:, :nt, :],
                        h_ap,
                        idx_sb[:, t0 * 8:t0 * 8 + nidx // 16],
                        num_idxs=nidx,
                        num_idxs_reg=nidx,
                        elem_size=D,
                        single_packet=False,
                        queue_num=qn % NQ,
                    )
                    qn += 1
                    gt0 = base_t + t0
                    oh = opool.tile([128, GCH, WND], BF16, tag="oh")
                    nc.vector.tensor_tensor(
                        oh[:, :nt, :],
                        dl_sb[:, gt0:gt0 + nt, None].to_broadcast([128, nt, WND]),
                        iota_sb[:, :nt, :],
                        mybir.AluOpType.is_equal,
                    )
                    nc.vector.tensor_tensor(
                        oh[:, :nt, :],
                        oh[:, :nt, :],
                        nse_sb[:, gt0:gt0 + nt, None].to_broadcast([128, nt, WND]),
                        mybir.AluOpType.mult,
                    )
                    for tl in range(nt):
                        t = gt0 + tl
                        wdw, k, Twh = tile_meta[t]
                        if k == 0:
                            psum = ppool.tile([128, WND], F32, tag="ps")
                        nc.tensor.matmul(
                            psum[:],
                            lhsT=g[:, tl, :],
                            rhs=oh[:, tl, :],
                            start=(k == 0),
                            stop=(k == Twh - 1),
                        )
                        if k == Twh - 1:
                            if not touched[wdw]:
                                nc.scalar.copy(aggW[wdw][:], psum[:])
                                touched[wdw] = True
                            else:
                                nc.vector.tensor_add(
                                    aggW[wdw][:], aggW[wdw][:], psum[:])
                            if is_b:
                                emit_output(wdw)

            for wdw in range(NW):
                if not touched[wdw]:
                    nc.vector.memset(aggW[wdw][:], 0.0)
                if not out_done[wdw]:
                    emit_output(wdw)

    nc.compile()
    return nc


def kernel(x, edge_index, W, b):
    global LAST_RESULTS
    x = np.asarray(x, dtype=np.float32)
    W = np.asarray(W, dtype=np.float32)
    b = np.asarray(b, dtype=np.float32)

    in_maps, tile_meta, ThA, ThB, bias_zero = _prep_inputs(x, edge_index, W, b)
    nc = _build_program(tile_meta, ThA, ThB, bias_zero)

    kwargs = {}
    if TRACE:
        kwargs["trace"] = True
    res = run_bass_kernel_spmd(nc, in_maps, list(range(NCORES)), **kwargs)
    LAST_RESULTS = res

    out = np.empty((N, D), dtype=np.float32)
    for c in range(NCORES):
        yc = np.asarray(res.results[c]["y"])          # [128, OCH, 128]
        rows = yc.transpose(1, 0, 2).reshape(OCH * 128, D)
        out[c * NPC:(c + 1) * NPC] = rows[:NPC]
    return out


# revision 21
# speedup vs baseline: 1.5103x; 1.3529x over previous
"""GCN block (DGL GraphConv norm='both' + ReLU) on 8 TRN2 NeuronCores.

Strategy (SPMD, one program for all cores; per-core data via inputs):
  - Nodes/edges sharded by destination: core c owns dst rows [c*6250, (c+1)*6250).
  - The gather table is the raw bf16 x (no normalized-table build pass):
    the src normalization rsqrt(deg_out) is folded into the one-hot rhs of
    the segment-sum matmul (DVE is_equal against an iota, then a batched
    DVE multiply by ns[src] per edge slot).
  - Edges sorted by (src half, dst window of 128). Per (window, half) group
    the tile count is the max over the 8 cores (SPMD uniform schedule);
    within a group each 128-edge tile does one matmul
    psum[128f, 128d] += g[128e, 128f]^T @ oh[128e, 128d].
  - After each group, psum is copied (Scalar engine) / added (DVE) into the
    per-window aggW tile. As soon as a window's last group completes, the
    output chunk runs inline: PE matmul aggW^T @ W, Scalar engine
    Relu(psum * rsqrt(deg_in)) (bias fused away when b == 0), and a
    per-window DMA of the 128 output rows, so the output stage fully
    overlaps the gather stream.

dma_gather indices are int16, so the table is split in two halves at row
32768; edges are grouped into two passes by source half. GpSimd descriptor
generation (~3 ns/idx, strictly serial on the Pool engine) is the
bottleneck; the schedule exists to keep every other engine hidden under it.
"""

import sys

if "/opt/trn_rl_repo" not in sys.path:
    sys.path.insert(0, "/opt/trn_rl_repo")

import numpy as np
import ml_dtypes

import concourse.bacc as bacc
import concourse.mybir as mybir
from concourse.bass import AP
from concourse.bass_utils import run_bass_kernel_spmd
from concourse.tile import TileContext

N = 50000          # nodes
D = 128            # feature dim
NCORES = 8
NPC = N // NCORES  # 6250 dst nodes per core

RN = 50048         # padded node count (multiple of 128)
HALF = 32768       # int16 index limit; table split [0, HALF) / [HALF, RN)

WND = 128                         # dst window width (= psum cols per group)
NW = (NPC + WND - 1) // WND       # 49 windows per core
OCH = NW                          # output chunks of 128 dst rows

GCH = 16                          # gather chunk: tiles per dma_gather call
NQ = 4                            # SWDGE queues used round-robin

F32 = mybir.dt.float32
BF16 = mybir.dt.bfloat16
I16 = mybir.dt.int16

TRACE = False            # set by test harness for profiling
LAST_RESULTS = None      # BassKernelResults of the last run


def _gather_idx_layout(vals):
    """[E] int16 -> [128, E//16] in dma_gather layout (16-wrap, 8x replicated)."""
    base = vals.reshape(-1, 16).T          # [16, E/16]
    return np.ascontiguousarray(np.tile(base, (8, 1)))


def _prep_inputs(x, edge_index, W, b):
    src = np.asarray(edge_index[0], dtype=np.int64)
    dst = np.asarray(edge_index[1], dtype=np.int64)
    E = src.shape[0]

    deg_out = np.bincount(src, minlength=N).astype(np.float64)
    deg_in = np.bincount(dst, minlength=N).astype(np.float64)
    ns = (1.0 / np.sqrt(np.maximum(deg_out, 1.0))).astype(np.float32)  # [N]
    nd = (1.0 / np.sqrt(np.maximum(deg_in, 1.0))).astype(np.float32)   # [N]

    core = dst // NPC
    dstl = dst - core * NPC
    half = (src >= HALF).astype(np.int64)
    w = dstl // WND

    # group id per (core, half, window); emit order is half-major
    gid = (core * 2 + half) * NW + w
    counts = np.bincount(gid, minlength=NCORES * 2 * NW).reshape(NCORES, 2 * NW)
    # uniform tiles per (half, window) group across cores
    T = np.maximum(0, -(-counts.max(axis=0) // 128)).astype(np.int64)  # [2*NW]
    tile_base = np.zeros(2 * NW + 1, dtype=np.int64)
    np.cumsum(T, out=tile_base[1:])
    TT = int(tile_base[-1])          # total tiles per core
    ThA = int(T[:NW].sum())          # tiles in half-0 pass
    ThB = TT - ThA

    # slot assignment: per core, edges ranked within their group
    order = np.argsort(gid, kind="stable")
    gid_s = gid[order]
    gstart = np.zeros(NCORES * 2 * NW + 1, dtype=np.int64)
    np.cumsum(counts.reshape(-1), out=gstart[1:])
    rank = np.arange(E, dtype=np.int64) - gstart[gid_s]

    core_s = core[order]
    slot = tile_base[gid_s - core_s * 2 * NW] * 128 + rank  # slot in core's schedule
    src_s = src[order]
    half_s = half[order]
    dl_s = (dstl - w * WND)[order]
    ns_s = ns[src_s]

    NSLOT = TT * 128
    idx_all = np.zeros((NCORES, NSLOT), dtype=np.int16)
    dl_all = np.full((NCORES, NSLOT), -1.0, dtype=np.float32)
    ns_all = np.zeros((NCORES, NSLOT), dtype=np.float32)
    idx_all[core_s, slot] = np.where(half_s == 0, src_s, src_s - HALF).astype(np.int16)
    dl_all[core_s, slot] = dl_s
    ns_all[core_s, slot] = ns_s
    dl_all = dl_all.astype(ml_dtypes.bfloat16)
    ns_all = ns_all.astype(ml_dtypes.bfloat16)

    # tile meta shared by all cores: (window, k within group, group size)
    tile_meta = []
    for g in range(2 * NW):
        for k in range(int(T[g])):
            tile_meta.append((g % NW, k, int(T[g])))

    bias_zero = bool(np.all(np.asarray(b) == 0.0))

    # replicated tensors
    xp = np.zeros((RN, D), dtype=ml_dtypes.bfloat16)
    xp[:N] = np.asarray(x, dtype=np.float32).astype(ml_dtypes.bfloat16)
    x_dev = np.ascontiguousarray(xp)

    W_dev = np.ascontiguousarray(np.asarray(W, dtype=np.float32))
    brep = np.ascontiguousarray(
        np.tile(np.asarray(b, dtype=np.float32)[None, :], (128, 1)))
    iota_rep = np.ascontiguousarray(np.broadcast_to(
        np.arange(WND, dtype=np.float32)[None, None, :], (128, GCH, WND)
    )).astype(ml_dtypes.bfloat16)

    in_maps = []
    for c in range(NCORES):
        ndp = np.zeros(OCH * 128, dtype=np.float32)
        ndp[:NPC] = nd[c * NPC:(c + 1) * NPC]
        nd_dev = np.ascontiguousarray(ndp.reshape(OCH, 128).T)  # [128, OCH]
        in_maps.append({
            "x_dev": x_dev,
            "ndr": nd_dev,
            "w": W_dev,
            "brep": brep,
            "iota_rep": iota_rep,
            "idx_a": _gather_idx_layout(idx_all[c, :ThA * 128]),
            "idx_b": _gather_idx_layout(idx_all[c, ThA * 128:]),
            # per tile: [128, TT], tile t partition p = edge slot t*128+p
            "dl": np.ascontiguousarray(dl_all[c].reshape(TT, 128).T),
            "nse16": np.ascontiguousarray(
                ns_all[c].astype(ml_dtypes.bfloat16).reshape(TT, 128).T),
        })
    return in_maps, tile_meta, ThA, ThB, bias_zero


def _build_program(tile_meta, ThA, ThB, bias_zero):
    TT = ThA + ThB

    nc = bacc.Bacc("TRN2", target_bir_lowering=False, debug=False,
                   num_devices=NCORES, num_swdge_queues=NQ)

    x_d = nc.dram_tensor("x_dev", [RN, D], BF16, kind="ExternalInput")
    ndr_d = nc.dram_tensor("ndr", [128, OCH], F32, kind="ExternalInput")
    w_d = nc.dram_tensor("w", [D, D], F32, kind="ExternalInput")
    brep_d = nc.dram_tensor("brep", [128, D], F32, kind="ExternalInput")
    iota_d = nc.dram_tensor("iota_rep", [128, GCH, WND], BF16, kind="ExternalInput")
    idx_a = nc.dram_tensor("idx_a", [128, ThA * 8], I16, kind="ExternalInput")
    idx_b = nc.dram_tensor("idx_b", [128, ThB * 8], I16, kind="ExternalInput")
    dl_d = nc.dram_tensor("dl", [128, TT], BF16, kind="ExternalInput")
    nse16_d = nc.dram_tensor("nse16", [128, TT], BF16, kind="ExternalInput")
    y_d = nc.dram_tensor("y", [128, OCH, D], F32, kind="ExternalOutput")

    with TileContext(nc) as tc:
        with (
            tc.tile_pool(name="const", bufs=1) as cpool,
            tc.tile_pool(name="gbuf", bufs=8) as gpool,
            tc.tile_pool(name="ohbuf", bufs=6) as opool,
            tc.tile_pool(name="agg", bufs=1) as apool,
            tc.tile_pool(name="psum", bufs=6, space="PSUM") as ppool,
            tc.tile_pool(name="psum2", bufs=2, space="PSUM") as ppool2,
        ):
            # ---- constants / small loads ----
            idx_a_sb = cpool.tile([128, ThA * 8], I16, tag="idxa")
            nc.sync.dma_start(out=idx_a_sb[:], in_=idx_a[:, :])
            idx_b_sb = cpool.tile([128, ThB * 8], I16, tag="idxb")
            nc.sync.dma_start(out=idx_b_sb[:], in_=idx_b[:, :])
            dl_sb = cpool.tile([128, TT], BF16, tag="dl")
            nc.sync.dma_start(out=dl_sb[:], in_=dl_d[:, :])
            nse16_sb = cpool.tile([128, TT], BF16, tag="nse16")
            nc.sync.dma_start(out=nse16_sb[:], in_=nse16_d[:, :])
            iota_sb = cpool.tile([128, GCH, WND], BF16, tag="iota")
            nc.sync.dma_start(out=iota_sb[:], in_=iota_d[:, :, :])
            w_sb = cpool.tile([D, D], F32, tag="w")
            nc.sync.dma_start(out=w_sb[:], in_=w_d[:, :])
            ndr_sb = cpool.tile([128, OCH], F32, tag="ndr")
            nc.sync.dma_start(out=ndr_sb[:], in_=ndr_d[:, :])
            if not bias_zero:
                brep_sb = cpool.tile([128, D], F32, tag="brep")
                nc.sync.dma_start(out=brep_sb[:], in_=brep_d[:, :])

            aggW = [apool.tile([128, WND], F32, tag=f"agg{w}", name=f"aggW{w}")
                    for w in range(NW)]
            outW = [apool.tile([128, D], F32, tag=f"out{w}", name=f"outW{w}")
                    for w in range(NW)]
            touched = [False] * NW
            out_done = [False] * NW

            def emit_output(wdw):
                ps2 = ppool2.tile([128, D], F32, tag="ps2")
                nc.tensor.matmul(
                    ps2[:],
                    lhsT=aggW[wdw][:],
                    rhs=w_sb[:],
                    start=True,
                    stop=True,
                )
                if bias_zero:
                    # out = relu(ps2 * nd), on the Scalar engine
                    nc.scalar.activation(
                        outW[wdw][:], ps2[:],
                        mybir.ActivationFunctionType.Relu,
                        bias=0.0, scale=ndr_sb[:, wdw:wdw + 1],
                    )
                else:
                    nc.vector.tensor_scalar(
                        outW[wdw][:], ps2[:], ndr_sb[:, wdw:wdw + 1], None,
                        mybir.AluOpType.mult,
                    )
                    nc.vector.tensor_tensor(
                        outW[wdw][:], outW[wdw][:], brep_sb[:],
                        mybir.AluOpType.add,
                    )
                    nc.vector.tensor_scalar_max(outW[wdw][:], outW[wdw][:], 0.0)
                nc.sync.dma_start(out=y_d[:, wdw, :], in_=outW[wdw][:])
                out_done[wdw] = True

            qn = 0
            psum = None
            for is_b, idx_sb, Th, base_t, h_ap in (
                (False, idx_a_sb, ThA, 0, x_d[0:HALF, :]),
                (True, idx_b_sb, ThB, ThA, x_d[HALF:RN, :]),
            ):
                for t0 in range(0, Th, GCH):
                    nt = min(GCH, Th - t0)
                    nidx = nt * 128
                    g = gpool.tile([128, GCH, D], BF16, tag="g")
                    nc.gpsimd.dma_gather(
                        g[:, :nt, :],
                        h_ap,
                        idx_sb[:, t0 * 8:t0 * 8 + nidx // 16],
                        num_idxs=nidx,
                        num_idxs_reg=nidx,
                        elem_size=D,
                        single_packet=False,
                        queue_num=qn % NQ,
                    )
                    qn += 1
                    gt0 = base_t + t0
                    oh = opool.tile([128, GCH, WND], BF16, tag="oh")
                    nc.vector.tensor_tensor(
                        oh[:, :nt, :],
                        dl_sb[:, gt0:gt0 + nt, None].to_broadcast([128, nt, WND]),
                        iota_sb[:, :nt, :],
                        mybir.AluOpType.is_equal,
                    )
                    nc.vector.tensor_tensor(
                        oh[:, :nt, :],
                        oh[:, :nt, :],
                        nse16_sb[:, gt0:gt0 + nt, None].to_broadcast(
                            [128, nt, WND]),
                        mybir.AluOpType.mult,
                    )
                    for tl in range(nt):
                        t = gt0 + tl
                        wdw, k, Twh = tile_meta[t]
                        if k == 0:
                            psum = ppool.tile([128, WND], F32, tag="ps")
                        nc.tensor.matmul(
                            psum[:],
                            lhsT=g[:, tl, :],
                            rhs=oh[:, tl, :],
                            start=(k == 0),
                            stop=(k == Twh - 1),
                        )
                        if k == Twh - 1:
                            if not touched[wdw]:
                                nc.scalar.copy(aggW[wdw][:], psum[:])
                                touched[wdw] = True
                            else:
                                nc.vector.tensor_add(
                                    aggW[wdw][:], aggW[wdw][:], psum[:])
                            if is_b:
                                emit_output(wdw)

            for wdw in range(NW):
                if not touched[wdw]:
                    nc.vector.memset(aggW[wdw][:], 0.0)
                if not out_done[wdw]:
                    emit_output(wdw)

    nc.compile()
    return nc


def kernel(x, edge_index, W, b):
    global LAST_RESULTS
    x = np.asarray(x, dtype=np.float32)
    W = np.asarray(W, dtype=np.float32)
    b = np.asarray(b, dtype=np.float32)

    in_maps, tile_meta, ThA, ThB, bias_zero = _prep_inputs(x, edge_index, W, b)
    nc = _build_program(tile_meta, ThA, ThB, bias_zero)

    kwargs = {}
    if TRACE:
        kwargs["trace"] = True
    res = run_bass_kernel_spmd(nc, in_maps, list(range(NCORES)), **kwargs)
    LAST_RESULTS = res

    out = np.empty((N, D), dtype=np.float32)
    for c in range(NCORES):
        yc = np.asarray(res.results[c]["y"])          # [128, OCH, 128]
        rows = yc.transpose(1, 0, 2).reshape(OCH * 128, D)
        out[c * NPC:(c + 1) * NPC] = rows[:NPC]
    return out
